# revision 47
# baseline (speedup 1.0000x reference)
"""Trainium2 Bass kernel for nn_DQNAgent_modify (dense_mlp) — fp8 DoubleRow.

Reference computation:
    q_before = mlp(obs.reshape(bs, -1))                      # raw obs
    pert[b, i, k] = obs_flat[b] - onehot(i) x feat[b, k]     # bs*2N rows
    q_after = mlp(pert / norm)                               # [bs, 2N]
    out = q_after - q_before                                 # [bs, 2N]

Two structural facts drive this version:

1. Layer-0 collapse (as in the f32r baseline): the perturbation touches
   only 4 of the 512 input features, so layer 0 of the big batch is
   z = base - corr with base = (obs/norm) @ W0a computed once per
   sample; both terms are matmuls against host-built operands (sel
   broadcast matrix + block-diagonal S from feat).

2. The output q_after - q_before is norm-dominated by q_before (RMS
   ~1.67 vs q_after's ~0.033), so the big 16384-row batch tolerates
   large relative error: quantizing every big-batch matmul to fp8-e4m3
   measures 3.4e-3 final rel err (gate 2e-2).  That unlocks
   MatmulPerfMode.DoubleRow: K=256 contracted per pass at 0.5
   cycles/output-row — ~4x f32r PE throughput.  Only the 64-row
   q_before mini-MLP stays f32r.

With the PE thus accelerated the bottleneck is PSUM->SBUF relu+cast
evictions, so: psum tiles are [128, 2(chunk), 512] pairs evicted in one
op (per-mt bias stays a legal per-partition scalar), evictions rotate
across ACT/DVE/Pool by a weighted load balancer, the final Wv result is
DMA'd straight from PSUM, and b0a rides for free in the selector slot
(sel row 64 = ones, base row 64 = b0a).  W2b (K=128) uses a DoubleRow
pass whose two slots are the two chunks' h5 with zero-padded weight
variants, so it too runs at 256 cycles.

Sharding: pure data parallel over batch, 64 samples/core on 8 cores;
weights replicated.  Row order on device is r = (g, i_lo, k, b) with
i = 32g + i_lo; the host unpermutes and applies q_after - q_before.
"""

import numpy as np
import ml_dtypes

import concourse.mybir as mybir
import concourse.tile as tile
from concourse import bacc
from concourse.bass_utils import run_bass_kernel_spmd

N_CORES = 8
BS, N, D = 512, 128, 4
BSL = BS // N_CORES        # 64 samples per core
IN = N * D                 # 512 input features
NG = 4                     # i-groups == 128-row blocks of W0a
NT = 8                     # 512-row chunks per group
NCHUNK = NG * NT           # 32 chunks of 512 rows per core
F32 = mybir.dt.float32
F32R = mybir.dt.float32r
BF16 = mybir.dt.bfloat16
F8 = mybir.dt.float8e4
NP_F8 = ml_dtypes.float8_e4m3
NP_BF16 = ml_dtypes.bfloat16
DR = mybir.MatmulPerfMode.DoubleRow
RELU = mybir.ActivationFunctionType.Relu
COPY = mybir.ActivationFunctionType.Copy
ADD = mybir.AluOpType.add
MAX = mybir.AluOpType.max

# fp8 DR-packed dense layers: (name, KT2 = K/256, M)
DR_LAYERS = [("W0b", 1, 512), ("W1a", 2, 512), ("W1b", 2, 256),
             ("W2a", 1, 128)]
BIAS_OF = {"W0b": "b0b", "W1a": "b1a", "W1b": "b1b", "W2a": "b2a",
           "W2b": "b2b"}
# f32r weights for the q_before mini-MLP
WSHAPES = [("W0a", IN, 256), ("W0b", 256, 512), ("W1a", 512, 512),
           ("W1b", 512, 256), ("W2a", 256, 128), ("W2b", 128, 256),
           ("Wv", 256, 1)]
QB_LAYERS = [("W0b", 2, 4), ("W1a", 4, 4), ("W1b", 4, 2), ("W2a", 2, 1),
             ("W2b", 1, 2)]
BSHAPES = [("b0a", 2), ("b0b", 4), ("b1a", 4), ("b1b", 2), ("b2a", 1),
           ("b2b", 2)]

_CACHE = {}


def _build():
    nc = bacc.Bacc("TRN2", target_bir_lowering=False, debug=False,
                   num_devices=N_CORES)

    dram = {}
    # bf16 qb weights: q_before tolerates ~0.3% weight error (output
    # contribution ~3e-3 rel) and bf16 halves 2.8 MB of head-window DMA
    # AND streams 1 cyc/row on the PE vs small-ap f32r's 2
    for name, kd, md in WSHAPES:
        dram[name] = nc.dram_tensor(name, [kd, md], BF16,
                                    kind="ExternalInput").ap()
    for name, kt2, md in DR_LAYERS:
        dram[name + "8"] = nc.dram_tensor(name + "8", [128, kt2 * 2 * md],
                                          F8, kind="ExternalInput").ap()
    # W2b dual-variant: [128, (var 2, slot 2, 256)]
    dram["W2b8"] = nc.dram_tensor("W2b8", [128, 1024], F8,
                                  kind="ExternalInput").ap()
    # [128, (slot 2, var 2, 16)]: variant v has Wv in column v of its
    # 16-wide block, zeros elsewhere; 16-wide blocks keep the DR
    # ldweights slot stride 16B-aligned (s3_lw_dual_fp8_restrictions).
    # Variant v routes chunk (p+v)'s Wv product to psum partition v, so
    # a pair accumulates into one [16, 512] region of a regular ps-ring
    # slot and is evicted as a cheap [2, 512] (512-col) copy instead of
    # a [1, 1024] single-partition crawl.
    dram["Wv8"] = nc.dram_tensor("Wv8", [128, 64], F8,
                                 kind="ExternalInput").ap()
    # W0a fp8 blocks for L0: [128, (g 4, mt 2, 128)]
    dram["l0w8"] = nc.dram_tensor("l0w8", [128, 1024], F8,
                                  kind="ExternalInput").ap()
    # zpad8: rows 64..127 of the base slot for all 4 g-packs
    # (row 64 = b0a, rest 0)
    dram["zpad8"] = nc.dram_tensor("zpad8", [64, 1024], F8,
                                   kind="ExternalInput").ap()
    # sel8 [128, (2, 512)]: the L0 selector (row 64 = ones for b0a),
    # duplicated so a col-offset AP covers both dci halves.  S8
    # [128, (cpair 4, 1024)]: per chunk PAIR the block-diagonal S from
    # feat.  On-device they land as two [sel, S_even, S_odd] tiles —
    # sel is shared (saves 2x128KB of head DMA) and an L0 matmul for an
    # odd cpair reads slots (0, 2) via a step-2 slot slice.
    dram["sel8"] = nc.dram_tensor("sel8", [128, 1024], F8,
                                  kind="ExternalInput").ap()
    dram["S8"] = nc.dram_tensor("S8", [128, 4096], F8,
                                kind="ExternalInput").ap()
    # obsU (raw obs for q_before, f32r)
    dram["bundleR"] = nc.dram_tensor("bundleR", [128, 256], BF16,
                                     kind="ExternalInput").ap()
    # obsS8 [128, (kt 4, 64)]: obs/norm in fp8 — base = obsS8 @ W0a runs
    # as fp8 DR so the 640KB f32r obsS/W0a DMA leaves the critical path
    dram["obsS8"] = nc.dram_tensor("obsS8", [128, 256], F8,
                                   kind="ExternalInput").ap()
    dram["bundleF"] = nc.dram_tensor("bundleF", [128, 15], F32,
                                     kind="ExternalInput").ap()
    qa_dram = nc.dram_tensor("qa", [1, NCHUNK * 512], F32,
                             kind="ExternalOutput").ap()
    qb_dram = nc.dram_tensor("qb", [1, BSL], F32, kind="ExternalOutput").ap()

    with tile.TileContext(nc) as tc:
        with (
            tc.tile_pool(name="wpool", bufs=1) as wpool,
            tc.tile_pool(name="cpool", bufs=1) as cpool,
            tc.tile_pool(name="hpool", bufs=3) as hpool,
            tc.tile_pool(name="qpool", bufs=3) as qpool,
            tc.tile_pool(name="ps", bufs=4, space="PSUM") as ps,
        ):
            # ---------------- setup DMAs ----------------
            # Three parallel DMA queues: sync-HWDGE (~75 KB/us),
            # scalar-HWDGE (~75 KB/us, free until the first eviction
            # lands ~11us in) and the slower gpsimd SWDGE (~46 KB/us).
            # Per-queue order = first-use deadline; the scalar queue
            # carries only head-critical fp8 operands so its ~0.7us
            # issue costs land before ACT's first eviction work.
            obsS8 = cpool.tile([128, 4, 64], F8, name="obsS8")
            nc.sync.dma_start(obsS8.rearrange("p k b -> p (k b)"),
                              dram["obsS8"])
            w0a8 = cpool.tile([128, 4, 2, 128], F8, name="w0a8")
            nc.sync.dma_start(w0a8.rearrange("p g mt m -> p (g mt m)"),
                              dram["l0w8"])

            # l0pack: [128, g 4, mt 2, slot 2, 128]; slot0 = base (+b0a
            # row), slot1 = W0a fp8 block — one tile, two DMA triggers
            l0pack_t = cpool.tile([128, 4, 2, 2, 128], F8, name="l0pack")
            nc.gpsimd.dma_start(
                l0pack_t[:, :, :, 1, :],
                dram["l0w8"].rearrange("p (g mt m) -> p g mt m", g=4, mt=2))
            nc.gpsimd.dma_start(
                l0pack_t[64:128, :, :, 0, :],
                dram["zpad8"].rearrange("p (g mt m) -> p g mt m", g=4, mt=2))
            l0pack = [l0pack_t[:, g, :, :, :] for g in range(4)]

            # ss tiles: [sel, S_even, S_odd]; cpair cp reads slots
            # (0, 1 + cp % 2) — a step-2 slot slice for odd cp
            ss1 = cpool.tile([128, 3, 1024], F8, name="ss1")
            ss2 = cpool.tile([128, 3, 1024], F8, name="ss2")
            nc.sync.dma_start(ss1[:, 0, :], dram["sel8"][:, :])
            nc.scalar.dma_start(ss1[:, 1, :], dram["S8"][:, 0:1024])
            nc.sync.dma_start(ss1[:, 2, :], dram["S8"][:, 1024:2048])
            bundleF = cpool.tile([128, 15], F32, name="bundleF")
            nc.gpsimd.dma_start(bundleF[:, :], dram["bundleF"][:, :])
            nc.sync.dma_start(ss2[:, 0, :], dram["sel8"][:, :])
            nc.scalar.dma_start(ss2[:, 1, :], dram["S8"][:, 2048:3072])
            nc.scalar.dma_start(ss2[:, 2, :], dram["S8"][:, 3072:4096])

            # fp8 DR weights, spread across queues in first-use order
            w8 = {}
            for name, kt2, md in DR_LAYERS:
                w8[name] = wpool.tile([128, kt2, 2, md], F8,
                                      name=f"w8_{name}")
            w8["W2b"] = wpool.tile([128, 2, 2, 256], F8, name="w8_W2b")
            w8["Wv"] = wpool.tile([128, 2, 2, 16], F8, name="w8_Wv")

            def _w8dma(eng, name):
                eng.dma_start(
                    w8[name].rearrange("p a s m -> p (a s m)")
                    if name != "Wv" else
                    w8["Wv"].rearrange("p s v m -> p (s v m)"),
                    dram[name + "8"])

            _w8dma(nc.scalar, "W0b")
            _w8dma(nc.gpsimd, "Wv")
            _w8dma(nc.sync, "W1a")
            _w8dma(nc.gpsimd, "W2a")
            _w8dma(nc.sync, "W1b")
            _w8dma(nc.scalar, "W2b")

            # f32r weights for q_before: 3.4 MB but not needed until the
            # first qb stage (~30 us in) — queued after all fp8 operands
            w_r = {}
            bundleR = cpool.tile([128, 256], BF16, name="bundleR")
            nc.gpsimd.dma_start(bundleR[:, :], dram["bundleR"][:, :])
            obsU_r = bundleR.rearrange("p (k b) -> p k b", k=4)
            w_r["W0a"] = wpool.tile([128, 4, 256], BF16, name="wr_W0a")
            for k in range(4):
                eng = (nc.sync, nc.gpsimd)[k % 2]
                eng.dma_start(w_r["W0a"][:, k, :],
                              dram["W0a"][128 * k:128 * (k + 1), :])
            for idx, (name, kd, md) in enumerate(WSHAPES[1:]):
                kt = kd // 128
                wr = wpool.tile([128, kt, md], BF16, name=f"wr_{name}")
                eng = (nc.gpsimd, nc.sync)[idx % 2]
                eng.dma_start(
                    wr[:, :, :],
                    dram[name].rearrange("(k p) m -> p k m", p=128))
                w_r[name] = wr

            b_off = {}
            off = 0
            for name, ntc in BSHAPES:
                b_off[name] = off
                off += ntc
            b_sb = {name: bundleF[:, b_off[name]:b_off[name] + ntc]
                    for name, ntc in BSHAPES}

            # ---- base = (obs/norm) @ W0a in fp8 DR (64 rows), evicted
            # as fp8 into slot 0 of all four l0pack tiles
            pbase = ps.tile([BSL, 256], F32, name="ps_base", tag="ps")
            for j in range(2):
                nc.tensor.matmul(
                    pbase[:, :], obsS8[:, 2 * j:2 * j + 2, :],
                    w0a8[:, 2 * j:2 * j + 2, :, :]
                    .rearrange("p k mt m -> p k (mt m)"),
                    start=(j == 0), stop=(j == 1), perf_mode=DR)
            # 2 ACT + 2 DVE: ACT's head DMA issues finish right as these
            # become runnable
            for g in range(4):
                if g % 2 == 0:
                    nc.scalar.activation(
                        l0pack[g][0:BSL, :, 0, :],
                        pbase[:, :].rearrange("b (mt m) -> b mt m", mt=2),
                        COPY)
                else:
                    nc.vector.tensor_copy(
                        l0pack[g][0:BSL, :, 0, :],
                        pbase[:, :].rearrange("b (mt m) -> b mt m", mt=2))

            # ------------- weighted eviction scheduler -------------
            # approximate per-op engine costs (ns) incl. fixed overhead
            # GPSIMD cannot access PSUM on TRN2 -> ACT + DVE only
            ev_load = {"act": 0.0, "dve": 0.0}

            def _pick(els):
                costs = {"act": els * 0.83 + 206,
                         "dve": els * 1.04 + 137}
                eng = min(ev_load, key=lambda e: ev_load[e] + costs[e])
                ev_load[eng] += costs[eng]
                return eng

            def evict(out_ap, psum_ap, bias_ap, els):
                b = bias_ap if bias_ap is not None else 0.0
                if _pick(els) == "act":
                    nc.scalar.activation(out_ap, psum_ap, RELU, bias=b)
                else:
                    nc.vector.tensor_scalar(out_ap, psum_ap, b, 0.0, ADD, MAX)

            def evict_split(out_ap, psum_ap, bias_ap):
                """Evict a [128, 2, 512] tile as two parallel [128, 512]
                halves, one per engine: halves the tile's eviction
                LATENCY so psum-ring reuse unblocks sooner (the ring
                stalls behind W0b/W1a eviction backlog otherwise)."""
                b = bias_ap if bias_ap is not None else 0.0
                ev_load["act"] += 512 * 0.83 + 206
                ev_load["dve"] += 512 * 1.04 + 137
                nc.scalar.activation(out_ap[:, 0:1, :], psum_ap[:, 0:1, :],
                                     RELU, bias=b)
                nc.vector.tensor_scalar(out_ap[:, 1:2, :], psum_ap[:, 1:2, :],
                                        b, 0.0, ADD, MAX)

            def evict_copy(out_ap, psum_ap, els):
                if _pick(els) == "act":
                    nc.scalar.activation(out_ap, psum_ap, COPY)
                else:
                    nc.vector.tensor_copy(out_ap, psum_ap)

            # ------- q_before mini-MLP (bf16), one layer per pair -------
            # each stage's evictions get a full pair (~9 us) to complete
            # before the next stage's matmuls need them, so the in-order
            # PE queue never blocks on the ACT/DVE queues.  All of a
            # stage's mt outputs carve disjoint [128, 64] regions out of
            # ONE regular ps-ring slot, so a stage costs a single ring
            # allocation (the baseline cost one per mt).
            qb_state = {}

            def _qb_region(pb, mt):
                return pb[:, mt // 2, 64 * (mt % 2):64 * (mt % 2) + 64]

            def qb_stage0():
                hq = []
                pb = ps.tile([128, 2, 512], F32, name="ps_qb0", tag="ps")
                for mt in range(2):
                    for kt in range(4):
                        nc.tensor.matmul(
                            _qb_region(pb, mt),
                            w_r["W0a"][:, kt, 128 * mt:128 * (mt + 1)],
                            obsU_r[:, kt, :],
                            start=(kt == 0), stop=(kt == 3))
                    h = qpool.tile([128, BSL], BF16, name=f"hq0_{mt}",
                                   tag=f"hq_{mt}")
                    evict(h[:, :], _qb_region(pb, mt),
                          b_sb["b0a"][:, mt:mt + 1], BSL)
                    hq.append(h)
                qb_state["hq"] = hq

            def qb_stage_mid(li):
                wname, ktn, mtn = QB_LAYERS[li]
                hq = qb_state.pop("hq")
                nxt = []
                pb = ps.tile([128, 2, 512], F32, name=f"ps_qb{li+1}",
                             tag="ps")
                for mt in range(mtn):
                    for kt in range(ktn):
                        nc.tensor.matmul(
                            _qb_region(pb, mt),
                            w_r[wname][:, kt, 128 * mt:128 * (mt + 1)],
                            hq[kt][:, :],
                            start=(kt == 0), stop=(kt == ktn - 1))
                    h = qpool.tile([128, BSL], BF16, name=f"hq{li+1}_{mt}",
                                   tag=f"hq_{mt}")
                    evict(h[:, :], _qb_region(pb, mt),
                          b_sb[BIAS_OF[wname]][:, mt:mt + 1], BSL)
                    nxt.append(h)
                qb_state["hq"] = nxt

            def qb_stage_out():
                hq = qb_state.pop("hq")
                pqb = ps.tile([128, 2, 512], F32, name="ps_qb_out",
                              tag="ps")
                for kt in range(2):
                    nc.tensor.matmul(pqb[0:1, 0, 0:BSL], w_r["Wv"][:, kt, :],
                                     hq[kt][:, :],
                                     start=(kt == 0), stop=(kt == 1))
                qb_sb = qpool.tile([1, BSL], F32, name="qb_sb")
                evict_copy(qb_sb[:, :], pqb[0:1, 0, 0:BSL], BSL)
                nc.sync.dma_start(qb_dram[:, :], qb_sb[:, :])

            qb_stages = ([qb_stage0]
                         + [lambda li=i: qb_stage_mid(li) for i in range(5)]
                         + [qb_stage_out])

            # ---------------- big-batch layers (fp8 DR) ----------------
            def emit_l0(ci0):
                """L0 for a chunk pair -> h1pair [128, mtK 2, chunk 2, 512]"""
                g = ci0 // NT
                h1 = hpool.tile([128, 2, 2, 512], F8, name=f"h1_{ci0}",
                                tag=f"h1_{ci0 % 4}", bufs=2)
                cp = (ci0 % NT) // 2
                sst = ss1 if cp < 2 else ss2
                ssl = sst[:, 0:2, :] if cp % 2 == 0 else sst[:, 0:3:2, :]
                for mt in range(2):
                    pp = ps.tile([128, 2, 512], F32, name=f"ps0_{ci0}_{mt}",
                                 tag="ps")
                    for dci in range(2):
                        nc.tensor.matmul(pp[:, dci, :],
                                         l0pack[g][:, mt, :, :],
                                         ssl[:, :, 512 * dci:512 * dci + 512],
                                         start=True, stop=True, perf_mode=DR)
                    # b0a folded via sel row 64 -> pure relu
                    evict(h1[:, mt, :, :], pp[:, :, :], None, 1024)
                return h1

            # super-pairs of 2 chunk-pairs: within each layer, pair P+1's
            # matmuls cover the eviction latency of pair P's outputs, so
            # the PE never waits at a layer boundary.
            h1_pending = {0: emit_l0(0), 2: emit_l0(2)}
            for si in range(0, NCHUNK, 4):
                pairs = (si, si + 2)
                h_cur = {p: h1_pending.pop(p) for p in pairs}
                for wname, kt2, md in DR_LAYERS:
                    mtn = md // 128
                    for p in pairs:
                        h = h_cur[p]
                        hn = hpool.tile([128, mtn, 2, 512], F8,
                                        name=f"h_{wname}_{p}",
                                        tag=f"h_{wname}_{p % 4}", bufs=2)
                        for mt in range(mtn):
                            pp = ps.tile([128, 2, 512], F32,
                                         name=f"ps_{wname}_{p}_{mt}",
                                         tag="ps")
                            for dci in range(2):
                                for j in range(kt2):
                                    nc.tensor.matmul(
                                        pp[:, dci, :],
                                        w8[wname][:, j, :,
                                                  128 * mt:128 * (mt + 1)],
                                        h[:, 2 * j:2 * j + 2, dci, :],
                                        start=(j == 0), stop=(j == kt2 - 1),
                                        perf_mode=DR)
                            if wname == "W1a" and p == si + 2:
                                # second pair's W1a tiles gate the psum
                                # ring for the following W1b allocs —
                                # halve their eviction latency
                                evict_split(
                                    hn[:, mt, :, :], pp[:, :, :],
                                    b_sb[BIAS_OF[wname]][:, mt:mt + 1])
                            else:
                                evict(hn[:, mt, :, :], pp[:, :, :],
                                      b_sb[BIAS_OF[wname]][:, mt:mt + 1],
                                      1024)
                        h_cur[p] = hn
                    # q_before stage after W1a: its slow-cadence PE work
                    # (LDW-bound small matmuls, few ring allocs) gives
                    # the eviction queues time to drain the W0b+W1a
                    # burst backlog before W1b's ring reuse
                    if wname == "W1a" and si >= 4:
                        qb_stages[si // 4 - 1]()
                # pull next super-pair's L0 here: its evictions precede
                # the tail's in the engine queues, and the tail's PE work
                # covers their latency
                if si + 4 < NCHUNK:
                    h1_pending[si + 4] = emit_l0(si + 4)
                    h1_pending[si + 6] = emit_l0(si + 6)
                h6s = {}
                for p in pairs:
                    # W2b: DR slots = the pair's two chunks of h5,
                    # zero-padded weight variants select one each
                    h5 = h_cur[p]
                    h6 = hpool.tile([128, 2, 2, 512], F8, name=f"h6_{p}",
                                    tag=f"h6_{p % 4}", bufs=2)
                    for mt in range(2):
                        pp = ps.tile([128, 2, 512], F32,
                                     name=f"ps6_{p}_{mt}", tag="ps")
                        for dci in range(2):
                            nc.tensor.matmul(
                                pp[:, dci, :],
                                w8["W2b"][:, dci, :, 128 * mt:128 * (mt + 1)],
                                h5[:, 0, :, :],
                                start=True, stop=True, perf_mode=DR)
                        evict(h6[:, mt, :, :], pp[:, :, :],
                              b_sb["b2b"][:, mt:mt + 1], 1024)
                    h6s[p] = h6
                for p in pairs:
                    # Wv: K=256 DR; chunk p+v routes to psum partition v
                    # via its zero-padded weight variant, so the pair
                    # accumulates in one [16, 512] ring-slot region and
                    # is evicted as a cheap [2, 512] (512-col) copy.
                    pq = ps.tile([16, 512], F32, name=f"psq_{p}",
                                 tag="ps")
                    for dci in range(2):
                        nc.tensor.matmul(pq[:, :],
                                         w8["Wv"][:, :, dci, :],
                                         h6s[p][:, :, dci, :],
                                         start=(dci == 0),
                                         stop=(dci == 1),
                                         perf_mode=DR)
                    qa_sb = qpool.tile([2, 512], F32, name=f"qa_{p}",
                                       tag="qaev", bufs=3)
                    evict_copy(qa_sb[:, :], pq[0:2, :], 512)
                    nc.sync.dma_start(
                        qa_dram[0:1, 512 * p:512 * (p + 2)],
                        qa_sb[:, :])
    nc.compile()
    return nc


def get_nc():
    if "nc" not in _CACHE:
        _CACHE["nc"] = _build()
    return _CACHE["nc"]


def _pack_dr(W, kt2, md):
    """[K, M] f32 -> [128, kt2*2*md] fp8 DR layout (k = (j, slot, p))."""
    W8 = W.astype(NP_F8)
    return np.ascontiguousarray(
        W8.reshape(kt2, 2, 128, md).transpose(2, 0, 1, 3).reshape(128, -1))


def make_in_maps(obs, feat, W0a, b0a, W0b, b0b, W1a, b1a, W1b, b1b,
                 W2a, b2a, W2b, b2b, Wv, bv):
    obs = np.ascontiguousarray(obs, np.float32)
    feat = np.ascontiguousarray(feat, np.float32)
    norm = np.where(np.arange(IN) % 2 == 0, 42.0, 160.0).astype(np.float32)
    nd = norm[:D]
    W0a = np.ascontiguousarray(W0a, np.float32)

    w2b8 = np.zeros((128, 2, 2, 256), NP_F8)
    w2b8[:, 0, 0, :] = np.asarray(W2b, np.float32).astype(NP_F8)
    w2b8[:, 1, 1, :] = w2b8[:, 0, 0, :]
    wv8 = np.zeros((128, 2, 2, 16), NP_F8)
    wv_col = np.asarray(Wv, np.float32).reshape(2, 128).astype(NP_F8).T
    for v in range(2):
        wv8[:, :, v, v] = wv_col
    l0w8 = np.ascontiguousarray(
        W0a.astype(NP_F8).reshape(4, 128, 2, 128).transpose(1, 0, 2, 3)
        .reshape(128, -1))
    zpad8 = np.zeros((64, 4, 256), NP_F8)
    zpad8[0, :, :] = np.asarray(b0a, np.float32).astype(NP_F8)
    zpad8 = zpad8.reshape(64, 1024)

    shared = {
        "W0a": np.ascontiguousarray(W0a.astype(NP_BF16)),
        "W0b": np.ascontiguousarray(np.asarray(W0b, np.float32).astype(NP_BF16)),
        "W1a": np.ascontiguousarray(np.asarray(W1a, np.float32).astype(NP_BF16)),
        "W1b": np.ascontiguousarray(np.asarray(W1b, np.float32).astype(NP_BF16)),
        "W2a": np.ascontiguousarray(np.asarray(W2a, np.float32).astype(NP_BF16)),
        "W2b": np.ascontiguousarray(np.asarray(W2b, np.float32).astype(NP_BF16)),
        "Wv": np.ascontiguousarray(
            np.asarray(Wv, np.float32).reshape(256, 1).astype(NP_BF16)),
        "W0b8": _pack_dr(np.asarray(W0b, np.float32), 1, 512),
        "W1a8": _pack_dr(np.asarray(W1a, np.float32), 2, 512),
        "W1b8": _pack_dr(np.asarray(W1b, np.float32), 2, 256),
        "W2a8": _pack_dr(np.asarray(W2a, np.float32), 1, 128),
        "W2b8": np.ascontiguousarray(w2b8.reshape(128, -1)),
        "Wv8": np.ascontiguousarray(wv8.reshape(128, -1)),
        "l0w8": l0w8,
        "zpad8": zpad8,
        "bundleF": np.ascontiguousarray(np.concatenate(
            [np.asarray(b, np.float32).reshape(ntc, 128).T
             for b, ntc in [(b0a, 2), (b0b, 4), (b1a, 4), (b1b, 2),
                            (b2a, 1), (b2b, 2)]], axis=1)),      # [128, 15]
    }
    # sel: rows 0..63 broadcast base over (i_lo, k); row 64 = ones (b0a)
    sel = np.zeros((128, 512), np.float32)
    sel[:BSL, :] = np.tile(np.eye(BSL, dtype=np.float32), (1, 512 // BSL))
    sel[64, :] = 1.0

    obs_flat = obs.reshape(BS, IN)
    in_maps = []
    for cidx in range(N_CORES):
        sl = slice(cidx * BSL, (cidx + 1) * BSL)
        obsS = (obs_flat[sl] / norm).T.reshape(4, 128, BSL)
        obsS = obsS.transpose(1, 0, 2).reshape(128, 4 * BSL)
        obsU = obs_flat[sl].T.reshape(4, 128, BSL)
        obsU = obsU.transpose(1, 0, 2).reshape(128, 4 * BSL)

        # S[4*il+d, 128*il + k*64 + b] = -feat[b, k, d] / nd[d]
        fs = -(feat[sl] / nd)                      # [64, 2, 4]
        fsT = fs.transpose(2, 1, 0).reshape(D, 2 * BSL)
        S = np.zeros((128, 4096), np.float32)
        for il in range(32):
            S[4 * il:4 * il + 4, 128 * il:128 * (il + 1)] = fsT

        m = dict(shared)
        m["bundleR"] = np.ascontiguousarray(obsU.astype(NP_BF16))  # [128, 256]
        m["obsS8"] = np.ascontiguousarray(obsS.astype(NP_F8))    # [128, 256]
        m["sel8"] = np.ascontiguousarray(
            np.tile(sel, (1, 2)).astype(NP_F8))                  # [128, 1024]
        m["S8"] = np.ascontiguousarray(S.astype(NP_F8))          # [128, 4096]
        in_maps.append(m)
    return in_maps


def assemble(results):
    qa = np.stack([r["qa"].reshape(-1) for r in results])   # [8, 16384]
    qb = np.stack([r["qb"].reshape(-1) for r in results])   # [8, 64]
    # r = (g, i_lo, k, b) -> j = g*64 + i_lo*2 + k
    qa = qa.reshape(N_CORES, NG, 32, 2, BSL).transpose(0, 4, 1, 2, 3)
    qa = np.ascontiguousarray(qa).reshape(BS, 2 * N)
    return (qa - qb.reshape(BS, 1)).astype(np.float32)


def kernel(**inputs):
    nc = get_nc()
    in_maps = make_in_maps(**inputs)
    res = run_bass_kernel_spmd(nc, in_maps, core_ids=list(range(N_CORES)))
    return assemble(res.results)



# revision 49
# speedup vs baseline: 1.0258x; 1.0258x over previous
"""Trainium2 Bass kernel for nn_DQNAgent_modify (dense_mlp) — fp8 DoubleRow.

Reference computation:
    q_before = mlp(obs.reshape(bs, -1))                      # raw obs
    pert[b, i, k] = obs_flat[b] - onehot(i) x feat[b, k]     # bs*2N rows
    q_after = mlp(pert / norm)                               # [bs, 2N]
    out = q_after - q_before                                 # [bs, 2N]

Two structural facts drive this version:

1. Layer-0 collapse (as in the f32r baseline): the perturbation touches
   only 4 of the 512 input features, so layer 0 of the big batch is
   z = base - corr with base = (obs/norm) @ W0a computed once per
   sample; both terms are matmuls against host-built operands (sel
   broadcast matrix + block-diagonal S from feat).

2. The output q_after - q_before is norm-dominated by q_before (RMS
   ~1.67 vs q_after's ~0.033), so the big 16384-row batch tolerates
   large relative error: quantizing every big-batch matmul to fp8-e4m3
   measures 3.4e-3 final rel err (gate 2e-2).  That unlocks
   MatmulPerfMode.DoubleRow: K=256 contracted per pass at 0.5
   cycles/output-row — ~4x f32r PE throughput.  Only the 64-row
   q_before mini-MLP stays f32r.

With the PE thus accelerated the bottleneck is PSUM->SBUF relu+cast
evictions, so: psum tiles are [128, 2(chunk), 512] pairs evicted in one
op (per-mt bias stays a legal per-partition scalar), evictions rotate
across ACT/DVE/Pool by a weighted load balancer, the final Wv result is
DMA'd straight from PSUM, and b0a rides for free in the selector slot
(sel row 64 = ones, base row 64 = b0a).  W2b (K=128) uses a DoubleRow
pass whose two slots are the two chunks' h5 with zero-padded weight
variants, so it too runs at 256 cycles.

Sharding: pure data parallel over batch, 64 samples/core on 8 cores;
weights replicated.  Row order on device is r = (g, i_lo, k, b) with
i = 32g + i_lo; the host unpermutes and applies q_after - q_before.
"""

import numpy as np
import ml_dtypes

import concourse.mybir as mybir
import concourse.tile as tile
from concourse import bacc
from concourse.bass_utils import run_bass_kernel_spmd

N_CORES = 8
BS, N, D = 512, 128, 4
BSL = BS // N_CORES        # 64 samples per core
IN = N * D                 # 512 input features
NG = 4                     # i-groups == 128-row blocks of W0a
NT = 8                     # 512-row chunks per group
NCHUNK = NG * NT           # 32 chunks of 512 rows per core
F32 = mybir.dt.float32
F32R = mybir.dt.float32r
BF16 = mybir.dt.bfloat16
F8 = mybir.dt.float8e4
NP_F8 = ml_dtypes.float8_e4m3
NP_BF16 = ml_dtypes.bfloat16
DR = mybir.MatmulPerfMode.DoubleRow
RELU = mybir.ActivationFunctionType.Relu
COPY = mybir.ActivationFunctionType.Copy
ADD = mybir.AluOpType.add
MAX = mybir.AluOpType.max

# fp8 DR-packed dense layers: (name, KT2 = K/256, M)
DR_LAYERS = [("W0b", 1, 512), ("W1a", 2, 512), ("W1b", 2, 256),
             ("W2a", 1, 128)]
BIAS_OF = {"W0b": "b0b", "W1a": "b1a", "W1b": "b1b", "W2a": "b2a",
           "W2b": "b2b"}
# f32r weights for the q_before mini-MLP
WSHAPES = [("W0a", IN, 256), ("W0b", 256, 512), ("W1a", 512, 512),
           ("W1b", 512, 256), ("W2a", 256, 128), ("W2b", 128, 256),
           ("Wv", 256, 1)]
QB_LAYERS = [("W0b", 2, 4), ("W1a", 4, 4), ("W1b", 4, 2), ("W2a", 2, 1),
             ("W2b", 1, 2)]
BSHAPES = [("b0a", 2), ("b0b", 4), ("b1a", 4), ("b1b", 2), ("b2a", 1),
           ("b2b", 2)]

_CACHE = {}


def _build():
    nc = bacc.Bacc("TRN2", target_bir_lowering=False, debug=False,
                   num_devices=N_CORES)

    dram = {}
    # bf16 qb weights: q_before tolerates ~0.3% weight error (output
    # contribution ~3e-3 rel) and bf16 halves 2.8 MB of head-window DMA
    # AND streams 1 cyc/row on the PE vs small-ap f32r's 2
    for name, kd, md in WSHAPES:
        dram[name] = nc.dram_tensor(name, [kd, md], BF16,
                                    kind="ExternalInput").ap()
    for name, kt2, md in DR_LAYERS:
        dram[name + "8"] = nc.dram_tensor(name + "8", [128, kt2 * 2 * md],
                                          F8, kind="ExternalInput").ap()
    # W2b dual-variant: [128, (var 2, slot 2, 256)]
    dram["W2b8"] = nc.dram_tensor("W2b8", [128, 1024], F8,
                                  kind="ExternalInput").ap()
    # [128, (slot 2, var 2, 16)]: variant v has Wv in column v of its
    # 16-wide block, zeros elsewhere; 16-wide blocks keep the DR
    # ldweights slot stride 16B-aligned (s3_lw_dual_fp8_restrictions).
    # Variant v routes chunk (p+v)'s Wv product to psum partition v, so
    # a pair accumulates into one [16, 512] region of a regular ps-ring
    # slot and is evicted as a cheap [2, 512] (512-col) copy instead of
    # a [1, 1024] single-partition crawl.
    dram["Wv8"] = nc.dram_tensor("Wv8", [128, 64], F8,
                                 kind="ExternalInput").ap()
    # W0a fp8 blocks for L0: [128, (g 4, mt 2, 128)]
    dram["l0w8"] = nc.dram_tensor("l0w8", [128, 1024], F8,
                                  kind="ExternalInput").ap()
    # zpad8: rows 64..127 of the base slot for all 4 g-packs
    # (row 64 = b0a, rest 0)
    dram["zpad8"] = nc.dram_tensor("zpad8", [64, 1024], F8,
                                   kind="ExternalInput").ap()
    # sel8 [128, (2, 512)]: the L0 selector (row 64 = ones for b0a),
    # duplicated so a col-offset AP covers both dci halves.  S8
    # [128, (cpair 4, 1024)]: per chunk PAIR the block-diagonal S from
    # feat.  On-device they land as two [sel, S_even, S_odd] tiles —
    # sel is shared (saves 2x128KB of head DMA) and an L0 matmul for an
    # odd cpair reads slots (0, 2) via a step-2 slot slice.
    dram["sel8"] = nc.dram_tensor("sel8", [128, 1024], F8,
                                  kind="ExternalInput").ap()
    dram["S8"] = nc.dram_tensor("S8", [128, 4096], F8,
                                kind="ExternalInput").ap()
    # obsU (raw obs for q_before, f32r)
    dram["bundleR"] = nc.dram_tensor("bundleR", [128, 256], BF16,
                                     kind="ExternalInput").ap()
    # obsS8 [128, (kt 4, 64)]: obs/norm in fp8 — base = obsS8 @ W0a runs
    # as fp8 DR so the 640KB f32r obsS/W0a DMA leaves the critical path
    dram["obsS8"] = nc.dram_tensor("obsS8", [128, 256], F8,
                                   kind="ExternalInput").ap()
    dram["bundleF"] = nc.dram_tensor("bundleF", [128, 15], F32,
                                     kind="ExternalInput").ap()
    qa_dram = nc.dram_tensor("qa", [1, NCHUNK * 512], F32,
                             kind="ExternalOutput").ap()
    qb_dram = nc.dram_tensor("qb", [1, BSL], F32, kind="ExternalOutput").ap()

    with tile.TileContext(nc) as tc:
        with (
            tc.tile_pool(name="wpool", bufs=1) as wpool,
            tc.tile_pool(name="cpool", bufs=1) as cpool,
            tc.tile_pool(name="hpool", bufs=3) as hpool,
            tc.tile_pool(name="qpool", bufs=3) as qpool,
            tc.tile_pool(name="ps", bufs=4, space="PSUM") as ps,
        ):
            # ---------------- setup DMAs ----------------
            # Three parallel DMA queues: sync-HWDGE (~75 KB/us),
            # scalar-HWDGE (~75 KB/us, free until the first eviction
            # lands ~11us in) and the slower gpsimd SWDGE (~46 KB/us).
            # Per-queue order = first-use deadline; the scalar queue
            # carries only head-critical fp8 operands so its ~0.7us
            # issue costs land before ACT's first eviction work.
            obsS8 = cpool.tile([128, 4, 64], F8, name="obsS8")
            nc.sync.dma_start(obsS8.rearrange("p k b -> p (k b)"),
                              dram["obsS8"])
            w0a8 = cpool.tile([128, 4, 2, 128], F8, name="w0a8")
            nc.sync.dma_start(w0a8.rearrange("p g mt m -> p (g mt m)"),
                              dram["l0w8"])

            # l0pack: [128, g 4, mt 2, slot 2, 128]; slot0 = base (+b0a
            # row), slot1 = W0a fp8 block — one tile, two DMA triggers
            l0pack_t = cpool.tile([128, 4, 2, 2, 128], F8, name="l0pack")
            nc.gpsimd.dma_start(
                l0pack_t[:, :, :, 1, :],
                dram["l0w8"].rearrange("p (g mt m) -> p g mt m", g=4, mt=2))
            nc.gpsimd.dma_start(
                l0pack_t[64:128, :, :, 0, :],
                dram["zpad8"].rearrange("p (g mt m) -> p g mt m", g=4, mt=2))
            l0pack = [l0pack_t[:, g, :, :, :] for g in range(4)]

            # ss tiles: [sel, S_even, S_odd]; cpair cp reads slots
            # (0, 1 + cp % 2) — a step-2 slot slice for odd cp
            ss1 = cpool.tile([128, 3, 1024], F8, name="ss1")
            ss2 = cpool.tile([128, 3, 1024], F8, name="ss2")
            nc.sync.dma_start(ss1[:, 0, :], dram["sel8"][:, :])
            nc.scalar.dma_start(ss1[:, 1, :], dram["S8"][:, 0:1024])
            nc.sync.dma_start(ss1[:, 2, :], dram["S8"][:, 1024:2048])
            bundleF = cpool.tile([128, 15], F32, name="bundleF")
            nc.gpsimd.dma_start(bundleF[:, :], dram["bundleF"][:, :])
            nc.sync.dma_start(ss2[:, 0, :], dram["sel8"][:, :])
            nc.scalar.dma_start(ss2[:, 1, :], dram["S8"][:, 2048:3072])
            nc.scalar.dma_start(ss2[:, 2, :], dram["S8"][:, 3072:4096])

            # fp8 DR weights, spread across queues in first-use order
            w8 = {}
            for name, kt2, md in DR_LAYERS:
                w8[name] = wpool.tile([128, kt2, 2, md], F8,
                                      name=f"w8_{name}")
            w8["W2b"] = wpool.tile([128, 2, 2, 256], F8, name="w8_W2b")
            w8["Wv"] = wpool.tile([128, 2, 2, 16], F8, name="w8_Wv")

            def _w8dma(eng, name):
                eng.dma_start(
                    w8[name].rearrange("p a s m -> p (a s m)")
                    if name != "Wv" else
                    w8["Wv"].rearrange("p s v m -> p (s v m)"),
                    dram[name + "8"])

            _w8dma(nc.scalar, "W0b")
            _w8dma(nc.gpsimd, "Wv")
            _w8dma(nc.sync, "W1a")
            _w8dma(nc.gpsimd, "W2a")
            _w8dma(nc.sync, "W1b")
            _w8dma(nc.scalar, "W2b")

            # f32r weights for q_before: 3.4 MB but not needed until the
            # first qb stage (~30 us in) — queued after all fp8 operands
            w_r = {}
            bundleR = cpool.tile([128, 256], BF16, name="bundleR")
            nc.gpsimd.dma_start(bundleR[:, :], dram["bundleR"][:, :])
            obsU_r = bundleR.rearrange("p (k b) -> p k b", k=4)
            w_r["W0a"] = wpool.tile([128, 4, 256], BF16, name="wr_W0a")
            for k in range(4):
                eng = (nc.sync, nc.gpsimd)[k % 2]
                eng.dma_start(w_r["W0a"][:, k, :],
                              dram["W0a"][128 * k:128 * (k + 1), :])
            for idx, (name, kd, md) in enumerate(WSHAPES[1:]):
                kt = kd // 128
                wr = wpool.tile([128, kt, md], BF16, name=f"wr_{name}")
                eng = (nc.gpsimd, nc.sync)[idx % 2]
                eng.dma_start(
                    wr[:, :, :],
                    dram[name].rearrange("(k p) m -> p k m", p=128))
                w_r[name] = wr

            b_off = {}
            off = 0
            for name, ntc in BSHAPES:
                b_off[name] = off
                off += ntc
            b_sb = {name: bundleF[:, b_off[name]:b_off[name] + ntc]
                    for name, ntc in BSHAPES}

            # ---- base = (obs/norm) @ W0a in fp8 DR (64 rows), evicted
            # as fp8 into slot 0 of all four l0pack tiles
            pbase = ps.tile([BSL, 256], F32, name="ps_base", tag="ps")
            for j in range(2):
                nc.tensor.matmul(
                    pbase[:, :], obsS8[:, 2 * j:2 * j + 2, :],
                    w0a8[:, 2 * j:2 * j + 2, :, :]
                    .rearrange("p k mt m -> p k (mt m)"),
                    start=(j == 0), stop=(j == 1), perf_mode=DR)
            # 2 ACT + 2 DVE: ACT's head DMA issues finish right as these
            # become runnable
            for g in range(4):
                if g % 2 == 0:
                    nc.scalar.activation(
                        l0pack[g][0:BSL, :, 0, :],
                        pbase[:, :].rearrange("b (mt m) -> b mt m", mt=2),
                        COPY)
                else:
                    nc.vector.tensor_copy(
                        l0pack[g][0:BSL, :, 0, :],
                        pbase[:, :].rearrange("b (mt m) -> b mt m", mt=2))

            # ------------- weighted eviction scheduler -------------
            # approximate per-op engine costs (ns) incl. fixed overhead
            # GPSIMD cannot access PSUM on TRN2 -> ACT + DVE only
            ev_load = {"act": 0.0, "dve": 0.0}

            def _pick(els):
                costs = {"act": els * 0.83 + 206,
                         "dve": els * 1.04 + 137}
                eng = min(ev_load, key=lambda e: ev_load[e] + costs[e])
                ev_load[eng] += costs[eng]
                return eng

            def evict(out_ap, psum_ap, bias_ap, els):
                b = bias_ap if bias_ap is not None else 0.0
                if _pick(els) == "act":
                    nc.scalar.activation(out_ap, psum_ap, RELU, bias=b)
                else:
                    nc.vector.tensor_scalar(out_ap, psum_ap, b, 0.0, ADD, MAX)

            def evict_split(out_ap, psum_ap, bias_ap):
                """Evict a [128, 2, 512] tile as two parallel [128, 512]
                halves, one per engine: halves the tile's eviction
                LATENCY so psum-ring reuse unblocks sooner (the ring
                stalls behind W0b/W1a eviction backlog otherwise)."""
                b = bias_ap if bias_ap is not None else 0.0
                ev_load["act"] += 512 * 0.83 + 206
                ev_load["dve"] += 512 * 1.04 + 137
                nc.scalar.activation(out_ap[:, 0:1, :], psum_ap[:, 0:1, :],
                                     RELU, bias=b)
                nc.vector.tensor_scalar(out_ap[:, 1:2, :], psum_ap[:, 1:2, :],
                                        b, 0.0, ADD, MAX)

            def evict_copy(out_ap, psum_ap, els):
                if _pick(els) == "act":
                    nc.scalar.activation(out_ap, psum_ap, COPY)
                else:
                    nc.vector.tensor_copy(out_ap, psum_ap)

            # ------- q_before mini-MLP (bf16), one layer per pair -------
            # each stage's evictions get a full pair (~9 us) to complete
            # before the next stage's matmuls need them, so the in-order
            # PE queue never blocks on the ACT/DVE queues.  All of a
            # stage's mt outputs carve disjoint [128, 64] regions out of
            # ONE regular ps-ring slot, so a stage costs a single ring
            # allocation (the baseline cost one per mt).
            qb_state = {}

            def _qb_region(pb, mt):
                return pb[:, mt // 2, 64 * (mt % 2):64 * (mt % 2) + 64]

            def qb_stage0():
                hq = []
                pb = ps.tile([128, 2, 512], F32, name="ps_qb0", tag="ps")
                for mt in range(2):
                    for kt in range(4):
                        nc.tensor.matmul(
                            _qb_region(pb, mt),
                            w_r["W0a"][:, kt, 128 * mt:128 * (mt + 1)],
                            obsU_r[:, kt, :],
                            start=(kt == 0), stop=(kt == 3))
                    h = qpool.tile([128, BSL], BF16, name=f"hq0_{mt}",
                                   tag=f"hq_{mt}")
                    evict(h[:, :], _qb_region(pb, mt),
                          b_sb["b0a"][:, mt:mt + 1], BSL)
                    hq.append(h)
                qb_state["hq"] = hq

            def qb_stage_mid(li):
                wname, ktn, mtn = QB_LAYERS[li]
                hq = qb_state.pop("hq")
                nxt = []
                pb = ps.tile([128, 2, 512], F32, name=f"ps_qb{li+1}",
                             tag="ps")
                for mt in range(mtn):
                    for kt in range(ktn):
                        nc.tensor.matmul(
                            _qb_region(pb, mt),
                            w_r[wname][:, kt, 128 * mt:128 * (mt + 1)],
                            hq[kt][:, :],
                            start=(kt == 0), stop=(kt == ktn - 1))
                    h = qpool.tile([128, BSL], BF16, name=f"hq{li+1}_{mt}",
                                   tag=f"hq_{mt}")
                    evict(h[:, :], _qb_region(pb, mt),
                          b_sb[BIAS_OF[wname]][:, mt:mt + 1], BSL)
                    nxt.append(h)
                qb_state["hq"] = nxt

            def qb_stage_out():
                hq = qb_state.pop("hq")
                pqb = ps.tile([128, 2, 512], F32, name="ps_qb_out",
                              tag="ps")
                for kt in range(2):
                    nc.tensor.matmul(pqb[0:1, 0, 0:BSL], w_r["Wv"][:, kt, :],
                                     hq[kt][:, :],
                                     start=(kt == 0), stop=(kt == 1))
                qb_sb = qpool.tile([1, BSL], F32, name="qb_sb")
                evict_copy(qb_sb[:, :], pqb[0:1, 0, 0:BSL], BSL)
                nc.sync.dma_start(qb_dram[:, :], qb_sb[:, :])

            qb_stages = ([qb_stage0]
                         + [lambda li=i: qb_stage_mid(li) for i in range(5)]
                         + [qb_stage_out])

            # ---------------- big-batch layers (fp8 DR) ----------------
            def emit_l0(ci0):
                """L0 for a chunk pair -> h1pair [128, mtK 2, chunk 2, 512]"""
                g = ci0 // NT
                h1 = hpool.tile([128, 2, 2, 512], F8, name=f"h1_{ci0}",
                                tag=f"h1_{ci0 % 4}", bufs=2)
                cp = (ci0 % NT) // 2
                sst = ss1 if cp < 2 else ss2
                ssl = sst[:, 0:2, :] if cp % 2 == 0 else sst[:, 0:3:2, :]
                for mt in range(2):
                    pp = ps.tile([128, 2, 512], F32, name=f"ps0_{ci0}_{mt}",
                                 tag="ps")
                    for dci in range(2):
                        nc.tensor.matmul(pp[:, dci, :],
                                         l0pack[g][:, mt, :, :],
                                         ssl[:, :, 512 * dci:512 * dci + 512],
                                         start=True, stop=True, perf_mode=DR)
                    # b0a folded via sel row 64 -> pure relu
                    evict(h1[:, mt, :, :], pp[:, :, :], None, 1024)
                return h1

            # super-pairs of 2 chunk-pairs: within each layer, pair P+1's
            # matmuls cover the eviction latency of pair P's outputs, so
            # the PE never waits at a layer boundary.
            h1_pending = {0: emit_l0(0), 2: emit_l0(2)}
            for si in range(0, NCHUNK, 4):
                pairs = (si, si + 2)
                h_cur = {p: h1_pending.pop(p) for p in pairs}
                for wname, kt2, md in DR_LAYERS:
                    mtn = md // 128
                    for p in pairs:
                        h = h_cur[p]
                        hn = hpool.tile([128, mtn, 2, 512], F8,
                                        name=f"h_{wname}_{p}",
                                        tag=f"h_{wname}_{p % 4}", bufs=2)
                        for mt in range(mtn):
                            pp = ps.tile([128, 2, 512], F32,
                                         name=f"ps_{wname}_{p}_{mt}",
                                         tag="ps")
                            for dci in range(2):
                                for j in range(kt2):
                                    nc.tensor.matmul(
                                        pp[:, dci, :],
                                        w8[wname][:, j, :,
                                                  128 * mt:128 * (mt + 1)],
                                        h[:, 2 * j:2 * j + 2, dci, :],
                                        start=(j == 0), stop=(j == kt2 - 1),
                                        perf_mode=DR)
                            if wname == "W1a" and p == si + 2:
                                # second pair's W1a tiles gate the psum
                                # ring for the following W1b allocs —
                                # halve their eviction latency
                                evict_split(
                                    hn[:, mt, :, :], pp[:, :, :],
                                    b_sb[BIAS_OF[wname]][:, mt:mt + 1])
                            else:
                                evict(hn[:, mt, :, :], pp[:, :, :],
                                      b_sb[BIAS_OF[wname]][:, mt:mt + 1],
                                      1024)
                        h_cur[p] = hn
                # pull next super-pair's L0 here: its evictions precede
                # the tail's in the engine queues, and the tail's PE work
                # covers their latency
                if si + 4 < NCHUNK:
                    h1_pending[si + 4] = emit_l0(si + 4)
                    h1_pending[si + 6] = emit_l0(si + 6)
                h6s = {}
                for p in pairs:
                    # W2b: DR slots = the pair's two chunks of h5,
                    # zero-padded weight variants select one each
                    h5 = h_cur[p]
                    h6 = hpool.tile([128, 2, 2, 512], F8, name=f"h6_{p}",
                                    tag=f"h6_{p % 4}", bufs=2)
                    for mt in range(2):
                        pp = ps.tile([128, 2, 512], F32,
                                     name=f"ps6_{p}_{mt}", tag="ps")
                        for dci in range(2):
                            nc.tensor.matmul(
                                pp[:, dci, :],
                                w8["W2b"][:, dci, :, 128 * mt:128 * (mt + 1)],
                                h5[:, 0, :, :],
                                start=True, stop=True, perf_mode=DR)
                        evict(h6[:, mt, :, :], pp[:, :, :],
                              b_sb["b2b"][:, mt:mt + 1], 1024)
                    h6s[p] = h6
                # q_before stage: independent PE work that covers the h6
                # eviction latency before Wv reads it
                if si >= 4:
                    qb_stages[si // 4 - 1]()
                for p in pairs:
                    # Wv: K=256 DR; chunk p+v routes to psum partition v
                    # via its zero-padded weight variant, so the pair
                    # accumulates in one [16, 512] ring-slot region and
                    # is evicted as a cheap [2, 512] (512-col) copy.
                    pq = ps.tile([16, 512], F32, name=f"psq_{p}",
                                 tag="ps")
                    for dci in range(2):
                        nc.tensor.matmul(pq[:, :],
                                         w8["Wv"][:, :, dci, :],
                                         h6s[p][:, :, dci, :],
                                         start=(dci == 0),
                                         stop=(dci == 1),
                                         perf_mode=DR)
                    qa_sb = qpool.tile([2, 512], F32, name=f"qa_{p}",
                                       tag="qaev", bufs=3)
                    evict_copy(qa_sb[:, :], pq[0:2, :], 512)
                    nc.sync.dma_start(
                        qa_dram[0:1, 512 * p:512 * (p + 2)],
                        qa_sb[:, :])
    nc.compile()
    return nc


def get_nc():
    if "nc" not in _CACHE:
        _CACHE["nc"] = _build()
    return _CACHE["nc"]


def _pack_dr(W, kt2, md):
    """[K, M] f32 -> [128, kt2*2*md] fp8 DR layout (k = (j, slot, p))."""
    W8 = W.astype(NP_F8)
    return np.ascontiguousarray(
        W8.reshape(kt2, 2, 128, md).transpose(2, 0, 1, 3).reshape(128, -1))


def make_in_maps(obs, feat, W0a, b0a, W0b, b0b, W1a, b1a, W1b, b1b,
                 W2a, b2a, W2b, b2b, Wv, bv):
    obs = np.ascontiguousarray(obs, np.float32)
    feat = np.ascontiguousarray(feat, np.float32)
    norm = np.where(np.arange(IN) % 2 == 0, 42.0, 160.0).astype(np.float32)
    nd = norm[:D]
    W0a = np.ascontiguousarray(W0a, np.float32)

    w2b8 = np.zeros((128, 2, 2, 256), NP_F8)
    w2b8[:, 0, 0, :] = np.asarray(W2b, np.float32).astype(NP_F8)
    w2b8[:, 1, 1, :] = w2b8[:, 0, 0, :]
    wv8 = np.zeros((128, 2, 2, 16), NP_F8)
    wv_col = np.asarray(Wv, np.float32).reshape(2, 128).astype(NP_F8).T
    for v in range(2):
        wv8[:, :, v, v] = wv_col
    l0w8 = np.ascontiguousarray(
        W0a.astype(NP_F8).reshape(4, 128, 2, 128).transpose(1, 0, 2, 3)
        .reshape(128, -1))
    zpad8 = np.zeros((64, 4, 256), NP_F8)
    zpad8[0, :, :] = np.asarray(b0a, np.float32).astype(NP_F8)
    zpad8 = zpad8.reshape(64, 1024)

    shared = {
        "W0a": np.ascontiguousarray(W0a.astype(NP_BF16)),
        "W0b": np.ascontiguousarray(np.asarray(W0b, np.float32).astype(NP_BF16)),
        "W1a": np.ascontiguousarray(np.asarray(W1a, np.float32).astype(NP_BF16)),
        "W1b": np.ascontiguousarray(np.asarray(W1b, np.float32).astype(NP_BF16)),
        "W2a": np.ascontiguousarray(np.asarray(W2a, np.float32).astype(NP_BF16)),
        "W2b": np.ascontiguousarray(np.asarray(W2b, np.float32).astype(NP_BF16)),
        "Wv": np.ascontiguousarray(
            np.asarray(Wv, np.float32).reshape(256, 1).astype(NP_BF16)),
        "W0b8": _pack_dr(np.asarray(W0b, np.float32), 1, 512),
        "W1a8": _pack_dr(np.asarray(W1a, np.float32), 2, 512),
        "W1b8": _pack_dr(np.asarray(W1b, np.float32), 2, 256),
        "W2a8": _pack_dr(np.asarray(W2a, np.float32), 1, 128),
        "W2b8": np.ascontiguousarray(w2b8.reshape(128, -1)),
        "Wv8": np.ascontiguousarray(wv8.reshape(128, -1)),
        "l0w8": l0w8,
        "zpad8": zpad8,
        "bundleF": np.ascontiguousarray(np.concatenate(
            [np.asarray(b, np.float32).reshape(ntc, 128).T
             for b, ntc in [(b0a, 2), (b0b, 4), (b1a, 4), (b1b, 2),
                            (b2a, 1), (b2b, 2)]], axis=1)),      # [128, 15]
    }
    # sel: rows 0..63 broadcast base over (i_lo, k); row 64 = ones (b0a)
    sel = np.zeros((128, 512), np.float32)
    sel[:BSL, :] = np.tile(np.eye(BSL, dtype=np.float32), (1, 512 // BSL))
    sel[64, :] = 1.0

    obs_flat = obs.reshape(BS, IN)
    in_maps = []
    for cidx in range(N_CORES):
        sl = slice(cidx * BSL, (cidx + 1) * BSL)
        obsS = (obs_flat[sl] / norm).T.reshape(4, 128, BSL)
        obsS = obsS.transpose(1, 0, 2).reshape(128, 4 * BSL)
        obsU = obs_flat[sl].T.reshape(4, 128, BSL)
        obsU = obsU.transpose(1, 0, 2).reshape(128, 4 * BSL)

        # S[4*il+d, 128*il + k*64 + b] = -feat[b, k, d] / nd[d]
        fs = -(feat[sl] / nd)                      # [64, 2, 4]
        fsT = fs.transpose(2, 1, 0).reshape(D, 2 * BSL)
        S = np.zeros((128, 4096), np.float32)
        for il in range(32):
            S[4 * il:4 * il + 4, 128 * il:128 * (il + 1)] = fsT

        m = dict(shared)
        m["bundleR"] = np.ascontiguousarray(obsU.astype(NP_BF16))  # [128, 256]
        m["obsS8"] = np.ascontiguousarray(obsS.astype(NP_F8))    # [128, 256]
        m["sel8"] = np.ascontiguousarray(
            np.tile(sel, (1, 2)).astype(NP_F8))                  # [128, 1024]
        m["S8"] = np.ascontiguousarray(S.astype(NP_F8))          # [128, 4096]
        in_maps.append(m)
    return in_maps


def assemble(results):
    qa = np.stack([r["qa"].reshape(-1) for r in results])   # [8, 16384]
    qb = np.stack([r["qb"].reshape(-1) for r in results])   # [8, 64]
    # r = (g, i_lo, k, b) -> j = g*64 + i_lo*2 + k
    qa = qa.reshape(N_CORES, NG, 32, 2, BSL).transpose(0, 4, 1, 2, 3)
    qa = np.ascontiguousarray(qa).reshape(BS, 2 * N)
    return (qa - qb.reshape(BS, 1)).astype(np.float32)


def kernel(**inputs):
    nc = get_nc()
    in_maps = make_in_maps(**inputs)
    res = run_bass_kernel_spmd(nc, in_maps, core_ids=list(range(N_CORES)))
    return assemble(res.results)

